# revision 32
# baseline (speedup 1.0000x reference)
"""AdaptiveBiasReflectiveLayer kernel for 8 TRN2 NeuronCores (Bass/Tile).

Key algebra: every per-scale correction the reference applies is an [H]-vector
broadcast over all tokens (x_corr = x + c).  Projection statistics therefore
collapse to column moments of P = X @ proj.T:
    mu_s[r]    = s*w[r]*(Pbar[r] + (proj @ c)[r]) + bias[r]
    sigma_s[r] = s*w[r]*Pstd[r]            (variance is shift-invariant)
with proj @ c = Gp @ q, Gp = proj @ proj.T, c = q @ proj, and q a [R]
coefficient vector accumulated over applied scales.  So the device computes:
  A) P^T column sums / square-sums (one bf16 matmul over all tokens)
     plus per-token bn_stats of x
  B) an 8-core AllReduce of [128,4] stats + the tiny [R]-space decision chain
  C) out = LayerNorm(x + c) * gamma + beta, fused per 128-token tile.
"""

import numpy as np
import concourse.bass as bass
import concourse.bacc as bacc
import concourse.mybir as mybir
from concourse import tile
from concourse.bass_utils import run_bass_kernel_spmd

F32 = mybir.dt.float32
BF16 = mybir.dt.bfloat16
AF = mybir.ActivationFunctionType
OP = mybir.AluOpType

B, S, H, R = 4, 2048, 4096, 256
N_CORES = 8
NTOK = B * S                  # 8192 global tokens
NT = NTOK // N_CORES          # 1024 tokens per core
TILES = NT // 128             # 8 token tiles per core
HC = H // 128                 # 32 h-chunks
RC = R // 128                 # 2 r-chunks
EPS = 1e-6
ALPHA = 0.01
THR = 0.1 * (1.0 + 1.0)       # KL_THRESHOLD * (1 + VARIANCE_EMA)
SCALES = (1.0, 0.5, 0.1)

_CACHE = {}


def _build(triv_gamma: bool, triv_beta: bool):
    nc = bacc.Bacc("TRN2", target_bir_lowering=False, debug=False)

    x_ext = nc.dram_tensor("x", [NT, H], F32, kind="ExternalInput")
    proj_ext = nc.dram_tensor("proj", [R, H], F32, kind="ExternalInput")
    pb_ext = nc.dram_tensor("pbias", [128, RC], F32, kind="ExternalInput")
    rmu_ext = nc.dram_tensor("refmu", [128, RC], F32, kind="ExternalInput")
    rsig_ext = nc.dram_tensor("refsig", [128, RC], F32, kind="ExternalInput")
    pw_ext = nc.dram_tensor("pw", [128, 3 * RC], F32, kind="ExternalInput")
    gam_ext = nc.dram_tensor("gamma", [1, H], F32, kind="ExternalInput")
    bet_ext = nc.dram_tensor("beta", [1, H], F32, kind="ExternalInput")
    out_ext = nc.dram_tensor("out", [NT, H], F32, kind="ExternalOutput")

    st_in = nc.dram_tensor("st_in", [128, 2 * RC], F32)
    st_out = nc.dram_tensor("st_out", [128, 2 * RC], F32, addr_space="Shared")
    wu_in = nc.dram_tensor("wu_in", [1, 8], F32)
    wu_out = nc.dram_tensor("wu_out", [1, 8], F32, addr_space="Shared")

    with tile.TileContext(nc) as tc:
        with (
            tc.tile_pool(name="w", bufs=1) as pw,        # persistents
            tc.tile_pool(name="xt", bufs=2 if (triv_gamma and triv_beta) else 1) as pxt,      # transposed X blocks
            tc.tile_pool(name="big", bufs=2 if (triv_gamma and triv_beta) else 1) as pbig,    # [128,H] f32 staging/out
            tc.tile_pool(name="str", bufs=2 if (triv_gamma and triv_beta) else 1) as pstr,    # streaming bf16 tiles
            tc.tile_pool(name="sc", bufs=1) as psc,      # small scalar tiles
        ):
            # ---------- constants ----------
            ones_col = pw.tile([128, 1], F32, tag="ones_col")
            nc.vector.memset(ones_col[:], 1.0)
            ones_row = pw.tile([1, 128], F32, tag="ones_row")
            nc.vector.memset(ones_row[:], 1.0)
            ones_sq_bf = pw.tile([128, 128], BF16, tag="ones_sq_bf")
            nc.vector.memset(ones_sq_bf[:], 1.0)
            iota_row = pw.tile([128, 128], mybir.dt.int32, tag="iota_row")
            nc.gpsimd.iota(iota_row[:], pattern=[[1, 128]], base=0,
                           channel_multiplier=0)
            iota_rowf = pw.tile([128, 128], F32, tag="iota_rowf")
            nc.vector.tensor_copy(iota_rowf[:], iota_row[:])
            iota_col = pw.tile([128, 1], mybir.dt.int32, tag="iota_col")
            nc.gpsimd.iota(iota_col[:], pattern=[[0, 1]], base=0,
                           channel_multiplier=1)
            iota_colf = pw.tile([128, 1], F32, tag="iota_colf")
            nc.vector.tensor_copy(iota_colf[:], iota_col[:])
            ident = pw.tile([128, 128], BF16, tag="ident")
            nc.vector.tensor_scalar(
                out=ident[:], in0=iota_rowf[:], scalar1=iota_colf[:], scalar2=None,
                op0=OP.is_equal)

            def bcast(pps, scalar_sb, tag):
                """[1,1] f32 SBUF -> [128,1] f32 SBUF (PE broadcast)."""
                ps = pps.tile([128, 1], F32, tag="bc_ps", name="bc_ps")
                nc.tensor.matmul(ps[:], ones_row[:], scalar_sb[:],
                                 start=True, stop=True)
                sb = psc.tile([128, 1], F32, tag=tag, name=tag)
                nc.vector.tensor_copy(sb[:], ps[:])
                return sb

            def preduce_ps(pps, vec):
                """[128, RC] f32 -> [1,1] f32 PSUM sum over all R entries."""
                ps = pps.tile([1, 1], F32, tag="red_ps", name="red_ps", bufs=4)
                for c in range(RC):
                    nc.tensor.matmul(ps[:], vec[:, c:c + 1], ones_col[:],
                                     start=(c == 0), stop=(c == RC - 1))
                return ps

            def preduce(pps, vec, tag):
                ps = preduce_ps(pps, vec)
                sb = psc.tile([1, 1], F32, tag=tag, name=tag)
                nc.vector.tensor_copy(sb[:], ps[:])
                return sb

            # ---------- phase 0: warmup collective + weights ----------
            wut = psc.tile([1, 8], F32, tag="wut")
            nc.vector.memset(wut[:], 1.0)
            nc.sync.dma_start(wu_in[:], wut[:])
            nc.gpsimd.collective_compute(
                "AllReduce", OP.add,
                ins=[wu_in[:].opt()], outs=[wu_out[:].opt()],
                replica_groups=[list(range(N_CORES))])

            proj_bf = []
            for c in range(RC):
                t = pw.tile([128, H], BF16, tag=f"projbf{c}", name=f"projbf{c}")
                nc.gpsimd.dma_start(out=t[:], in_=proj_ext[c * 128:(c + 1) * 128, :])
                proj_bf.append(t)
            psA_cm = tc.tile_pool(name="psA", bufs=1, space="PSUM")
            psA = psA_cm.__enter__()

            projT = pw.tile([128, HC, R], BF16, tag="projT")
            for c in range(RC):
                for batch in range(4):
                    tp = psA.tile([128, 8, 128], BF16, tag="tp_ps",
                                  name="tp_ps", bufs=2)
                    for j in range(8):
                        hc = batch * 8 + j
                        nc.tensor.transpose(
                            tp[:, j, :],
                            proj_bf[c][:, hc * 128:(hc + 1) * 128], ident[:])
                    nc.vector.tensor_copy(
                        projT[:, batch * 8:(batch + 1) * 8,
                              c * 128:(c + 1) * 128].rearrange(
                                  "p a b -> p a b"),
                        tp[:])

            pb_sb = pw.tile([128, RC], F32, tag="pb")
            nc.sync.dma_start(pb_sb[:], pb_ext[:])
            rmu_sb = pw.tile([128, RC], F32, tag="rmu")
            nc.sync.dma_start(rmu_sb[:], rmu_ext[:])
            rsig_sb = pw.tile([128, RC], F32, tag="rsig")
            nc.sync.dma_start(rsig_sb[:], rsig_ext[:])
            pwts = pw.tile([128, 3 * RC], F32, tag="pwts")
            nc.sync.dma_start(pwts[:], pw_ext[:])
            w_all = pw.tile([128, 3 * RC], F32, tag="w_all")
            nc.scalar.activation(w_all[:], pwts[:], AF.Sigmoid)

            rsig_inv = pw.tile([128, RC], F32, tag="rsig_inv")
            nc.vector.reciprocal(rsig_inv[:], rsig_sb[:])
            rsig2 = pw.tile([128, RC], F32, tag="rsig2")
            nc.vector.tensor_mul(rsig2[:], rsig_sb[:], rsig_sb[:])

            # ---------- phase A: stream x, convert, bn_stats, transpose, matmul
            xbf = [pw.tile([128, H], BF16, tag=f"xbf{i}", name=f"xbf{i}")
                   for i in range(TILES)]
            # per-tile raw-x row sums, accumulated during the f32->bf16 convert
            sx = [psc.tile([128, 1], F32, tag=f"sx{i}", name=f"sx{i}")
                  for i in range(TILES)]
            PT_ps = [psA.tile([128, NT], F32, tag=f"pt{rt}", name=f"pt{rt}")
                     for rt in range(RC)]

            NBLK = TILES // 2    # 2-tile (256-token) XT blocks
            for b in range(NBLK):
                xt = pxt.tile([128, 2, HC, 128], BF16, tag="xt")
                for k in range(2):
                    i = 2 * b + k
                    stg = pbig.tile([128, H], F32, tag="bigf32", name="stg")
                    nc.sync.dma_start(stg[:], x_ext[i * 128:(i + 1) * 128, :])
                    nc.scalar.activation(xbf[i][:], stg[:], AF.Copy,
                                         accum_out=sx[i][:])
                    for batch in range(4):
                        tp = psA.tile([128, 8, 128], BF16, tag="tp_ps",
                                      name="tp_ps", bufs=2)
                        for j in range(8):
                            hc = batch * 8 + j
                            nc.tensor.transpose(
                                tp[:, j, :],
                                xbf[i][:, hc * 128:(hc + 1) * 128], ident[:])
                        nc.vector.tensor_copy(
                            xt[:, k, batch * 8:(batch + 1) * 8, :], tp[:])
                for rt in range(RC):
                    for hc in range(HC):
                        nc.tensor.matmul(
                            PT_ps[rt][:, b * 256:(b + 1) * 256],
                            projT[:, hc, rt * 128:(rt + 1) * 128],
                            xt[:, :, hc, :],
                            start=(hc == 0), stop=(hc == HC - 1))

            # column stats of P^T over local tokens
            stats_loc = psc.tile([128, 2 * RC], F32, tag="stats_loc")
            for rt in range(RC):
                nc.vector.tensor_reduce(
                    stats_loc[:, rt:rt + 1], PT_ps[rt][:],
                    axis=mybir.AxisListType.X, op=OP.add)
                dump = pstr.tile([128, NT], BF16, tag="sq_dump", name="sq_dump")
                nc.scalar.activation(
                    dump[:], PT_ps[rt][:], AF.Square,
                    accum_out=stats_loc[:, 2 + rt:2 + rt + 1])

            psA_cm.__exit__(None, None, None)

            # ---------- AllReduce ----------
            nc.sync.dma_start(st_in[:], stats_loc[:])
            nc.gpsimd.collective_compute(
                "AllReduce", OP.add,
                ins=[st_in[:].opt()], outs=[st_out[:].opt()],
                replica_groups=[list(range(N_CORES))])
            # Gp = proj @ proj.T  ->  Gp_sb[p, c1, r2] = Gp[c1*128+p, r2]
            psGp_cm = tc.tile_pool(name="psGp", bufs=1, space="PSUM")
            psGp = psGp_cm.__enter__()
            Gp_sb = pw.tile([128, RC, R], F32, tag="Gp")
            for c1 in range(RC):
                gps = psGp.tile([128, R], F32, tag="gp_ps", name="gp_ps")
                for hc in range(HC):
                    nc.tensor.matmul(
                        gps[:], projT[:, hc, c1 * 128:(c1 + 1) * 128],
                        projT[:, hc, :], start=(hc == 0), stop=(hc == HC - 1))
                nc.vector.tensor_copy(Gp_sb[:, c1, :], gps[:])



            psGp_cm.__exit__(None, None, None)
            stats_glb = psc.tile([128, 2 * RC], F32, tag="stats_glb")
            nc.sync.dma_start(stats_glb[:], st_out[:])

            # ---------- phase B: scalar chain ----------
            psB_cm = tc.tile_pool(name="psB", bufs=1, space="PSUM")
            psB = psB_cm.__enter__()

            Pbar = psc.tile([128, RC], F32, tag="Pbar")
            nc.vector.tensor_scalar_mul(Pbar[:], stats_glb[:, 0:RC], 1.0 / NTOK)
            EP2 = psc.tile([128, RC], F32, tag="EP2")
            nc.vector.tensor_scalar_mul(EP2[:], stats_glb[:, RC:2 * RC], 1.0 / NTOK)
            pb2 = psc.tile([128, RC], F32, tag="pb2")
            nc.vector.tensor_mul(pb2[:], Pbar[:], Pbar[:])
            Pvar = psc.tile([128, RC], F32, tag="Pvar")
            nc.vector.tensor_sub(Pvar[:], EP2[:], pb2[:])
            nc.vector.tensor_scalar_max(Pvar[:], Pvar[:], 0.0)
            Pstd = psc.tile([128, RC], F32, tag="Pstd")
            nc.scalar.activation(Pstd[:], Pvar[:], AF.Sqrt)

            def matvec(qv, tag):
                dps = psB.tile([128, RC], F32, tag="d_ps", name="d_ps")
                for c1 in range(RC):
                    for c2 in range(RC):
                        nc.tensor.matmul(
                            dps[:, c1:c1 + 1],
                            Gp_sb[:, c2, c1 * 128:(c1 + 1) * 128],
                            qv[:, c2:c2 + 1],
                            start=(c2 == 0), stop=(c2 == RC - 1))
                dsb = psc.tile([128, RC], F32, tag=tag, name=tag)
                nc.vector.tensor_copy(dsb[:], dps[:])
                return dsb

            def nt(tag, shape=(128, RC)):
                return psc.tile(list(shape), F32, tag=tag, name=tag)

            # decision-independent per-scale quantities, batched over scales:
            # sig3 = max(scale_s*w_s*Pstd, EPS); lg3 = ln(sig3/rsig + EPS)
            # is23 = 1/sig3^2; basev3 = lg3 + 0.5*rsig2*is23
            scl3 = pw.tile([128, 3 * RC], F32, tag="scl3")
            for s in range(3):
                nc.vector.memset(scl3[:, 2 * s:2 * s + 2], SCALES[s])
            pstd3 = nt("pstd3", (128, 3 * RC))
            for s in range(3):
                nc.vector.tensor_copy(pstd3[:, 2 * s:2 * s + 2], Pstd[:])
            rsi3 = nt("rsi3", (128, 3 * RC))
            rs23 = nt("rs23", (128, 3 * RC))
            eps3 = pw.tile([128, 3 * RC], F32, tag="eps3")
            nc.vector.memset(eps3[:], EPS)
            for s in range(3):
                nc.vector.tensor_copy(rsi3[:, 2 * s:2 * s + 2], rsig_inv[:])
                nc.vector.tensor_copy(rs23[:, 2 * s:2 * s + 2], rsig2[:])
            t33 = nt("t33", (128, 3 * RC))
            nc.vector.tensor_mul(t33[:], w_all[:], pstd3[:])
            nc.vector.tensor_mul(t33[:], t33[:], scl3[:])
            sig3 = nt("sig3", (128, 3 * RC))
            nc.vector.tensor_tensor(sig3[:], t33[:], eps3[:], OP.max)
            t43 = nt("t43", (128, 3 * RC))
            nc.vector.tensor_mul(t43[:], sig3[:], rsi3[:])
            nc.vector.tensor_scalar_add(t43[:], t43[:], EPS)
            lg3 = nt("lg3", (128, 3 * RC))
            nc.scalar.activation(lg3[:], t43[:], AF.Ln)
            s23 = nt("s23", (128, 3 * RC))
            nc.vector.tensor_mul(s23[:], sig3[:], sig3[:])
            is23 = nt("is23", (128, 3 * RC))
            nc.vector.reciprocal(is23[:], s23[:])
            b13 = nt("b13", (128, 3 * RC))
            nc.vector.tensor_mul(b13[:], rs23[:], is23[:])
            basev3 = nt("basev3", (128, 3 * RC))
            nc.vector.scalar_tensor_tensor(
                out=basev3[:], in0=b13[:], scalar=0.5, in1=lg3[:],
                op0=OP.mult, op1=OP.add)
            sbase_l = [preduce(psB, basev3[:, 2 * s:2 * s + 2], f"sbase{s}")
                       for s in range(3)]

            q = psc.tile([128, RC], F32, tag="q0")
            nc.vector.memset(q[:], 0.0)

            for s, scale in enumerate(SCALES):
                w_s = w_all[:, 2 * s:2 * s + 2]
                if s == 0:
                    d = nt(f"d{s}")
                    nc.vector.memset(d[:], 0.0)
                else:
                    d = matvec(q, f"d{s}")
                # mu = scale*w*(Pbar + d) + pb
                t1 = nt(f"t1_{s}")
                nc.vector.tensor_add(t1[:], Pbar[:], d[:])
                nc.vector.tensor_mul(t1[:], t1[:], w_s)
                mu = nt(f"mu{s}")
                nc.vector.scalar_tensor_tensor(
                    out=mu[:], in0=t1[:], scalar=scale, in1=pb_sb[:],
                    op0=OP.mult, op1=OP.add)
                is2 = is23[:, 2 * s:2 * s + 2]
                sbase = sbase_l[s]
                dm = nt(f"dm{s}")
                nc.vector.tensor_sub(dm[:], rmu_sb[:], mu[:])
                dm2 = nt(f"dm2_{s}")
                nc.vector.tensor_mul(dm2[:], dm[:], dm[:])
                g1 = nt(f"g1_{s}")
                nc.vector.tensor_mul(g1[:], dm2[:], is2)
                sg1 = preduce(psB, g1, f"sg1_{s}")
                skl = psc.tile([1, 1], F32, tag=f"skl{s}", name=f"skl{s}")
                nc.vector.scalar_tensor_tensor(
                    out=skl[:], in0=sg1[:], scalar=0.5, in1=sbase[:],
                    op0=OP.mult, op1=OP.add)
                a1 = psc.tile([1, 1], F32, tag=f"a1_{s}", name=f"a1_{s}")
                nc.vector.tensor_single_scalar(
                    a1[:], skl[:], R * (THR + 0.5), OP.is_gt)
                # adaptive alpha (negated):  -ALPHA*scale*clip(mean|dm|,.05,10)
                adm = nt(f"adm{s}")
                nc.scalar.activation(adm[:], dm[:], AF.Abs)
                absum = preduce_ps(psB, adm)
                aa = psc.tile([1, 1], F32, tag=f"aa{s}", name=f"aa{s}")
                nc.vector.tensor_scalar(
                    out=aa[:], in0=absum[:], scalar1=1.0 / R, scalar2=0.05,
                    op0=OP.mult, op1=OP.max)
                nc.vector.tensor_scalar(
                    out=aa[:], in0=aa[:], scalar1=10.0, scalar2=-ALPHA * scale,
                    op0=OP.min, op1=OP.mult)
                nsfb = bcast(psB, aa, f"nsfb{s}")
                # linearized post-mu: mu_post = mu + scale*nsfb*(w .* (Gp @ t6))
                t6 = nt(f"t6_{s}")
                nc.vector.tensor_mul(t6[:], dm[:], w_s)
                dd = matvec(t6, f"dd{s}")
                v1 = nt(f"v1_{s}")
                nc.vector.tensor_mul(v1[:], dd[:], w_s)
                v2 = nt(f"v2_{s}")
                nc.vector.tensor_scalar(
                    out=v2[:], in0=v1[:], scalar1=nsfb[:], scalar2=scale,
                    op0=OP.mult, op1=OP.mult)
                mup = nt(f"mup{s}")
                nc.vector.tensor_add(mup[:], mu[:], v2[:])
                dmp = nt(f"dmp{s}")
                nc.vector.tensor_sub(dmp[:], rmu_sb[:], mup[:])
                dmp2 = nt(f"dmp2_{s}")
                nc.vector.tensor_mul(dmp2[:], dmp[:], dmp[:])
                g2 = nt(f"g2_{s}")
                nc.vector.tensor_mul(g2[:], dmp2[:], is2)
                sg2 = preduce_ps(psB, g2)
                a2 = psc.tile([1, 1], F32, tag=f"a2_{s}", name=f"a2_{s}")
                nc.vector.tensor_tensor(a2[:], sg2[:], sg1[:], OP.is_lt)
                mask = psc.tile([1, 1], F32, tag=f"mask{s}", name=f"mask{s}")
                nc.vector.tensor_mul(mask[:], a1[:], a2[:])
                maskb = bcast(psB, mask, f"maskb{s}")
                mnb = psc.tile([128, 1], F32, tag=f"mnb{s}", name=f"mnb{s}")
                nc.vector.tensor_mul(mnb[:], maskb[:], nsfb[:])
                q_new = psc.tile([128, RC], F32, tag=f"q{s + 1}", name=f"q{s + 1}")
                nc.vector.scalar_tensor_tensor(
                    out=q_new[:], in0=t6[:], scalar=mnb[:], in1=q[:],
                    op0=OP.mult, op1=OP.add)
                q = q_new

            psB_cm.__exit__(None, None, None)

            # ---------- c_bcast = broadcast(q @ proj) as bf16 [128, H] ----------
            psC_cm = tc.tile_pool(name="psC", bufs=1, space="PSUM")
            psC = psC_cm.__enter__()
            q_rep = pw.tile([128, RC, 128], BF16, tag="q_rep")
            for c2 in range(RC):
                nc.vector.tensor_scalar_mul(
                    q_rep[:, c2, :], ones_sq_bf[:], q[:, c2:c2 + 1])
            cb_ps = psC.tile([128, H], F32, tag="cb_ps")
            for fc in range(H // 512):
                for c2 in range(RC):
                    nc.tensor.matmul(
                        cb_ps[:, fc * 512:(fc + 1) * 512],
                        q_rep[:, c2, :],
                        proj_bf[c2][:, fc * 512:(fc + 1) * 512],
                        start=(c2 == 0), stop=(c2 == RC - 1))
            c_bf = pw.tile([128, H], BF16, tag="c_bf")
            csum = psc.tile([128, 1], F32, tag="csum")
            nc.scalar.activation(c_bf[:], cb_ps[:], AF.Copy, accum_out=csum[:])
            mc = psc.tile([128, 1], F32, tag="mc")
            nc.vector.tensor_scalar_mul(mc[:], csum[:], 1.0 / H)
            psC_cm.__exit__(None, None, None)

            if not (triv_gamma and triv_beta):
                gam_row = pw.tile([1, H], F32, tag="gam_row")
                nc.sync.dma_start(gam_row[:], gam_ext[:])
                bet_row = pw.tile([1, H], F32, tag="bet_row")
                nc.sync.dma_start(bet_row[:], bet_ext[:])
                gb_cm = tc.tile_pool(name="psGB", bufs=1, space="PSUM")
                gbp = gb_cm.__enter__()
                gb_ps = gbp.tile([128, H], F32, tag="gb_ps")
                gam_rep = pw.tile([128, H], BF16, tag="gam_rep")
                bet_rep = pw.tile([128, H], BF16, tag="bet_rep")
                for fc in range(H // 512):
                    nc.tensor.matmul(gb_ps[:, fc * 512:(fc + 1) * 512],
                                     ones_row[:],
                                     gam_row[:, fc * 512:(fc + 1) * 512],
                                     start=True, stop=True)
                nc.vector.tensor_copy(gam_rep[:], gb_ps[:])
                for fc in range(H // 512):
                    nc.tensor.matmul(gb_ps[:, fc * 512:(fc + 1) * 512],
                                     ones_row[:],
                                     bet_row[:, fc * 512:(fc + 1) * 512],
                                     start=True, stop=True)
                nc.vector.tensor_copy(bet_rep[:], gb_ps[:])
                gb_cm.__exit__(None, None, None)

            # ---------- phase C: normalize ----------
            psD_cm = tc.tile_pool(name="psD", bufs=1, space="PSUM")
            psD = psD_cm.__enter__()
            for i in range(TILES):
                xc = pstr.tile([128, H], BF16, tag="xc", name="xc")
                nc.vector.tensor_add(xc[:], xbf[i][:], c_bf[:])
                negm = psc.tile([128, 1], F32, tag=f"negm{i}", name=f"negm{i}")
                nc.vector.scalar_tensor_tensor(
                    out=negm[:], in0=sx[i][:], scalar=-1.0 / H, in1=mc[:],
                    op0=OP.mult, op1=OP.subtract)
                sq_ps = psD.tile([128, H], F32, tag="sq_ps", name="sq_ps")
                ssq = psc.tile([128, 1], F32, tag=f"ssq{i}", name=f"ssq{i}")
                nc.scalar.activation(
                    sq_ps[:], xc[:], AF.Square, bias=negm[:], scale=1.0,
                    accum_out=ssq[:])
                var = psc.tile([128, 1], F32, tag=f"var{i}", name=f"var{i}")
                nc.vector.tensor_scalar(
                    out=var[:], in0=ssq[:], scalar1=1.0 / (H - 1), scalar2=0.0,
                    op0=OP.mult, op1=OP.max)
                std = psc.tile([128, 1], F32, tag=f"std{i}", name=f"std{i}")
                nc.scalar.activation(std[:], var[:], AF.Sqrt)
                nc.vector.tensor_scalar(
                    out=std[:], in0=std[:], scalar1=1e-5, scalar2=EPS,
                    op0=OP.max, op1=OP.add)
                kk = psc.tile([128, 1], F32, tag=f"kk{i}", name=f"kk{i}")
                nc.vector.reciprocal(kk[:], std[:])
                nmk = psc.tile([128, 1], F32, tag=f"nmk{i}", name=f"nmk{i}")
                nc.vector.tensor_mul(nmk[:], negm[:], kk[:])
                ot = pbig.tile([128, H], F32, tag="bigf32", name="ot")
                if i % 4 == 3:
                    nc.scalar.activation(
                        ot[:], xc[:], AF.Identity, bias=nmk[:], scale=kk[:])
                else:
                    nc.vector.tensor_scalar(
                        out=ot[:], in0=xc[:], scalar1=kk[:], scalar2=nmk[:],
                        op0=OP.mult, op1=OP.add)
                if not triv_gamma:
                    nc.vector.tensor_mul(ot[:], ot[:], gam_rep[:])
                if not triv_beta:
                    nc.vector.tensor_add(ot[:], ot[:], bet_rep[:])
                eng = nc.sync if i % 2 == 0 else nc.scalar
                eng.dma_start(out_ext[i * 128:(i + 1) * 128, :], ot[:])
            psD_cm.__exit__(None, None, None)

    nc.finalize()
    return nc


def _make_in_maps(inputs):
    x = np.ascontiguousarray(np.asarray(inputs["x"], dtype=np.float32))
    gamma = np.asarray(inputs["gamma"], dtype=np.float32)
    beta = np.asarray(inputs["beta"], dtype=np.float32)
    proj = np.ascontiguousarray(np.asarray(inputs["proj"], dtype=np.float32))
    Xf = x.reshape(NTOK, H)
    pb2c = np.ascontiguousarray(
        np.asarray(inputs["proj_bias"], np.float32).reshape(RC, 128).T)
    rmu2 = np.ascontiguousarray(
        np.asarray(inputs["ref_mu"], np.float32).reshape(RC, 128).T)
    rsg2 = np.ascontiguousarray(
        np.asarray(inputs["ref_sigma"], np.float32).reshape(RC, 128).T)
    pw6 = np.ascontiguousarray(
        np.asarray(inputs["proj_weights"], np.float32)
        .reshape(3, RC, 128).transpose(2, 0, 1).reshape(128, 3 * RC))
    return [{
        "x": np.ascontiguousarray(Xf[i * NT:(i + 1) * NT]),
        "proj": proj,
        "pbias": pb2c,
        "refmu": rmu2,
        "refsig": rsg2,
        "pw": pw6,
        "gamma": np.ascontiguousarray(gamma.reshape(1, H)),
        "beta": np.ascontiguousarray(beta.reshape(1, H)),
    } for i in range(N_CORES)]


def _get_nc(inputs):
    gamma = np.asarray(inputs["gamma"], dtype=np.float32)
    beta = np.asarray(inputs["beta"], dtype=np.float32)
    key = (bool(np.all(gamma == 1.0)), bool(np.all(beta == 0.0)))
    if key not in _CACHE:
        _CACHE[key] = _build(*key)
    return _CACHE[key]


def kernel(**inputs):
    nc = _get_nc(inputs)
    in_maps = _make_in_maps(inputs)
    res = run_bass_kernel_spmd(nc, in_maps, core_ids=list(range(N_CORES)))
    out = np.concatenate([res.results[i]["out"] for i in range(N_CORES)], axis=0)
    return out.reshape(B, S, H).astype(np.float32)
